# revision 6
# baseline (speedup 1.0000x reference)
"""DSA block kernel for Trainium2, data-parallel over batch on 8 NeuronCores.

Strategy notes (hardcoded to the [8,256,64,64] / temb [8,512] problem):
  - batch-parallel SPMD: core b computes batch element b entirely.
  - aafm: per-channel complex weights commute with fft2/ifft2, so
    ifft2(fft2(fg)*w1 + fft2(fl)*w2).real == w1r*fg + w2r*fl exactly.
    w1r is folded into the spc proj conv; w2r applied at the residual add.
  - topk-gather-attention: the output only depends on the selected channel
    SET (attention + scatter are permutation-equivariant), so it is computed
    as dense masked attention over all 256 channels: columns of non-selected
    channels get -1e30 before softmax, rows are zeroed after.
  - the top-k mask replicates lax.top_k tie-breaking exactly via
    rank_i = #{j: p_j > p_i} + #{j < i: p_j == p_i}; selected iff rank < 128.
  - matmuls run as float32r (fp32 storage, reduced-mantissa matmul at 1
    cycle/row) except the tiny temb/MLP matvecs which stay full fp32.
  - 1x1 convs are plain matmuls; the 3x3 convs accumulate 9 shifted taps in
    PSUM reading from a zero-padded [C, 66*66] layout; the depthwise 3x3
    uses diagonal weight matrices on the tensor engine.
"""

import os
import numpy as np

C = 256
H = W = 64
N = H * W           # 4096
TEMB = 512
HID = 512
GROUPS = 32
CPG = C // GROUPS   # 8
TK = 128            # top-k channels
EPS = 1e-6
PW = W + 2          # 66
HP = PW * PW        # 4356
NCORES = 8
SCALE = float(TK) ** -0.5
NELEM_G = CPG * N   # elements per group = 32768

DEBUG = os.environ.get("DSA_DEBUG", "0") == "1"

_CACHE = {}


def _build():
    import concourse.bacc as bacc
    import concourse.mybir as mybir
    from concourse.tile import TileContext
    from contextlib import ExitStack

    f32 = mybir.dt.float32
    f32r = mybir.dt.float32r
    AF = mybir.ActivationFunctionType
    ALU = mybir.AluOpType
    AX = mybir.AxisListType

    nc = bacc.Bacc("TRN2", target_bir_lowering=False, debug=False)

    def din(name, shape):
        return nc.dram_tensor(name, shape, f32, kind="ExternalInput").ap()

    XB = din("xb", [2, 128, N])
    TCOL = din("temb_col", [128, 4])

    TS_LHST = din("ts_lhsT", [4, 128, 1792])
    TS_BIAS = din("ts_bias", [14, 128, 1])
    M1_LHST = din("m1_lhsT", [4, 128, 512])
    M1_BIAS = din("m1_bias", [4, 128, 1])
    M2_LHST = din("m2_lhsT", [4, 128, 256])
    M2_BIAS = din("m2_bias", [2, 128, 1])
    WQ_T = din("wq_t", [2, 128, 256])
    WK_T = din("wk_t", [2, 128, 256])
    WV_T = din("wv_t", [2, 128, 256])
    WP_T = din("wp_t", [2, 128, 256])
    WPW_T = din("wpw_t", [2, 128, 256])
    WB_T = din("wb_t", [2, 128, 256])
    WA_T = din("wa_t", [9, 2, 128, 256])
    DWDIAG = din("dw_diag", [9, 2, 128, 128])
    BQ_ROW = din("bq_row", [1, 256])
    BK_ROW = din("bk_row", [1, 256])
    BV_COL = din("bv_col", [2, 128, 1])
    BP_COL = din("bp_col", [2, 128, 1])
    BPW_COL = din("bpw_col", [2, 128, 1])
    BB_COL = din("bb_col", [2, 128, 1])
    BA_ROW = din("ba_row", [2, 1, 128])
    BDW_COL = din("bdw_col", [2, 128, 1])
    W2R_COL = din("w2r_col", [2, 128, 1])
    GNG = din("g_ng", [2, 128, 1])
    BNG = din("b_ng", [2, 128, 1])
    GNL = din("g_nl", [2, 128, 1])
    BNL = din("b_nl", [2, 128, 1])
    GNF = din("g_nf", [2, 128, 1])
    BNF = din("b_nf", [2, 128, 1])
    GT = din("gmat_t", [2, 128, 32])
    GM = din("gmat", [32, 256])
    LTRI = din("ltri", [2, 128, 256])
    IDENT = din("ident", [128, 128])
    ONES1x128 = din("ones1x128", [1, 128])
    ONES1x512 = din("ones1x512", [1, 512])

    Y = nc.dram_tensor("y", [2, 128, N], f32, kind="ExternalOutput").ap()
    if DEBUG:
        DBG_PROMPT = nc.dram_tensor("dbg_prompt", [2, 128, 1], f32, kind="ExternalOutput").ap()
        DBG_MASK = nc.dram_tensor("dbg_mask", [2, 128, 1], f32, kind="ExternalOutput").ap()
        DBG_ATTN = nc.dram_tensor("dbg_attn", [2, 128, 256], f32, kind="ExternalOutput").ap()
        DBG_FG = nc.dram_tensor("dbg_fg", [2, 128, N], f32, kind="ExternalOutput").ap()
        DBG_FL = nc.dram_tensor("dbg_fl", [2, 128, N], f32, kind="ExternalOutput").ap()
        DBG_X1 = nc.dram_tensor("dbg_x1", [2, 128, N], f32, kind="ExternalOutput").ap()

    BOUNCE = nc.dram_tensor("bounce", [2, 256], f32)

    with TileContext(nc) as tc, ExitStack() as top:
        pw = top.enter_context(tc.tile_pool(name="pw", bufs=1))
        pco = top.enter_context(tc.tile_pool(name="pco", bufs=1))
        px = top.enter_context(tc.tile_pool(name="px", bufs=1))

        def wt(pool, name, src_ap, shape, dt):
            t = pool.tile(shape, dt, tag=name, name=name)
            if dt == f32:
                nc.sync.dma_start(t[:], src_ap)
            else:
                nc.gpsimd.dma_start(t[:], src_ap)
            return t

        wq = [wt(pw, f"wq{k}", WQ_T[k], [128, 256], f32r) for k in range(2)]
        wk = [wt(pw, f"wk{k}", WK_T[k], [128, 256], f32r) for k in range(2)]
        wv = [wt(pw, f"wv{k}", WV_T[k], [128, 256], f32r) for k in range(2)]
        wp = [wt(pw, f"wp{k}", WP_T[k], [128, 256], f32r) for k in range(2)]
        wpw = [wt(pw, f"wpw{k}", WPW_T[k], [128, 256], f32r) for k in range(2)]
        wb = [wt(pw, f"wb{k}", WB_T[k], [128, 256], f32r) for k in range(2)]
        dwd = [[wt(pw, f"dwd{t_}_{k}", DWDIAG[t_, k], [128, 128], f32r) for k in range(2)]
               for t_ in range(9)]
        bq_row = wt(pw, "bq_row", BQ_ROW[:], [1, 256], f32r)
        bk_row = wt(pw, "bk_row", BK_ROW[:], [1, 256], f32r)
        ba_row = [wt(pw, f"ba_row{m}", BA_ROW[m], [1, 128], f32r) for m in range(2)]
        ones128 = wt(pw, "ones128", ONES1x128[:], [1, 128], f32r)
        ones512 = wt(pw, "ones512", ONES1x512[:], [1, 512], f32r)
        ident = wt(pw, "ident", IDENT[:], [128, 128], f32r)
        gt = [wt(pw, f"gt{k}", GT[k], [128, 32], f32) for k in range(2)]
        gm = wt(pw, "gm", GM[:], [32, 256], f32)
        ltri = [wt(pw, f"ltri{k}", LTRI[k], [128, 256], f32) for k in range(2)]

        def col(name, src):
            return wt(pw, name, src, [128, 1], f32)

        bv = [col(f"bv{m}", BV_COL[m]) for m in range(2)]
        bp = [col(f"bp{m}", BP_COL[m]) for m in range(2)]
        bpw = [col(f"bpw{m}", BPW_COL[m]) for m in range(2)]
        bb = [col(f"bb{m}", BB_COL[m]) for m in range(2)]
        bdw = [col(f"bdw{m}", BDW_COL[m]) for m in range(2)]
        w2r = [col(f"w2r{m}", W2R_COL[m]) for m in range(2)]
        gng = [col(f"gng{m}", GNG[m]) for m in range(2)]
        bng = [col(f"bng{m}", BNG[m]) for m in range(2)]
        gnl = [col(f"gnl{m}", GNL[m]) for m in range(2)]
        bnl = [col(f"bnl{m}", BNL[m]) for m in range(2)]
        gnf = [col(f"gnf{m}", GNF[m]) for m in range(2)]
        bnf = [col(f"bnf{m}", BNF[m]) for m in range(2)]

        X = []
        for m in range(2):
            t = px.tile([128, N], f32, tag=f"x{m}", name=f"x{m}")
            nc.sync.dma_start(t[:], XB[m])
            X.append(t)

        def ctile(name):
            return pco.tile([128, 1], f32, tag=name, name=name)

        TT = nc.vector.tensor_tensor
        TS = nc.vector.tensor_scalar

        def sumsq_cols(src_tiles, names, sbpool, nbufs):
            out = []
            for m in range(2):
                scr = sbpool.tile([128, N], f32, tag="sqscr", name="sqscr", bufs=nbufs)
                acc = ctile(f"{names}{m}")
                nc.scalar.activation(scr[:], src_tiles[m][:], AF.Square,
                                     accum_out=acc[:])
                out.append(acc)
            return out

        def group_stats(sum_cols, ssq_cols, name, pspool):
            ps_s = pspool.tile([32, 1], f32, tag="grs", name="grs")
            ps_q = pspool.tile([32, 1], f32, tag="grq", name="grq")
            for k in range(2):
                nc.tensor.matmul(ps_s[:], gt[k][:], sum_cols[k][:],
                                 start=(k == 0), stop=(k == 1))
            for k in range(2):
                nc.tensor.matmul(ps_q[:], gt[k][:], ssq_cols[k][:],
                                 start=(k == 0), stop=(k == 1))
            mu_g = pco.tile([32, 1], f32, tag=f"mug_{name}", name=f"mug_{name}")
            TS(mu_g[:], ps_s[:], 1.0 / NELEM_G, None, ALU.mult)
            ex2 = pco.tile([32, 1], f32, tag=f"ex2_{name}", name=f"ex2_{name}")
            TS(ex2[:], ps_q[:], 1.0 / NELEM_G, None, ALU.mult)
            musq = pco.tile([32, 1], f32, tag=f"musq_{name}", name=f"musq_{name}")
            TT(musq[:], mu_g[:], mu_g[:], ALU.mult)
            vpe = pco.tile([32, 1], f32, tag=f"vpe_{name}", name=f"vpe_{name}")
            TT(vpe[:], ex2[:], musq[:], ALU.subtract)
            TS(vpe[:], vpe[:], EPS, None, ALU.add)
            sq = pco.tile([32, 1], f32, tag=f"sq_{name}", name=f"sq_{name}")
            nc.scalar.activation(sq[:], vpe[:], AF.Sqrt)
            r0 = pco.tile([32, 1], f32, tag=f"r0_{name}", name=f"r0_{name}")
            nc.vector.reciprocal(r0[:], sq[:])
            t1 = pco.tile([32, 1], f32, tag=f"t1_{name}", name=f"t1_{name}")
            TT(t1[:], r0[:], r0[:], ALU.mult)
            TT(t1[:], t1[:], vpe[:], ALU.mult)
            TS(t1[:], t1[:], -0.5, 1.5, ALU.mult, ALU.add)
            rr = pco.tile([32, 1], f32, tag=f"rr_{name}", name=f"rr_{name}")
            TT(rr[:], r0[:], t1[:], ALU.mult)
            mu_c, is_c = [], []
            for m in range(2):
                ps_m = pspool.tile([128, 1], f32, tag="grb", name="grb", bufs=2)
                nc.tensor.matmul(ps_m[:], gm[:, m * 128:(m + 1) * 128], mu_g[:],
                                 start=True, stop=True)
                mc = ctile(f"mu_{name}{m}")
                nc.vector.tensor_copy(mc[:], ps_m[:])
                ps_i = pspool.tile([128, 1], f32, tag="grb", name="grb", bufs=2)
                nc.tensor.matmul(ps_i[:], gm[:, m * 128:(m + 1) * 128], rr[:],
                                 start=True, stop=True)
                ic = ctile(f"is_{name}{m}")
                nc.vector.tensor_copy(ic[:], ps_i[:])
                mu_c.append(mc)
                is_c.append(ic)
            return mu_c, is_c

        def affine_coefs(gam, bet, mu_c, is_c, sc_cols, sh_cols, name):
            A, Bc = [], []
            for m in range(2):
                a1 = ctile(f"a1_{name}{m}")
                TT(a1[:], gam[m][:], is_c[m][:], ALU.mult)
                t = ctile(f"tp1_{name}{m}")
                TS(t[:], sc_cols[m][:], 1.0, None, ALU.add)
                a = ctile(f"A_{name}{m}")
                TT(a[:], a1[:], t[:], ALU.mult)
                b1 = ctile(f"b1_{name}{m}")
                TT(b1[:], mu_c[m][:], a1[:], ALU.mult)
                TT(b1[:], bet[m][:], b1[:], ALU.subtract)
                TT(b1[:], b1[:], t[:], ALU.mult)
                b = ctile(f"B_{name}{m}")
                TT(b[:], b1[:], sh_cols[m][:], ALU.add)
                A.append(a)
                Bc.append(b)
            return A, Bc

        # ======= phase A: temb projections, X stats, prompt MLP (fp32) ======
        with tc.tile_pool(name="pts", bufs=1) as pts, \
             tc.tile_pool(name="psmv", bufs=1, space="PSUM") as psmv:
            ts_w = [pts.tile([128, 1792], f32, tag=f"tsw{k}", name=f"tsw{k}") for k in range(4)]
            for k in range(4):
                nc.sync.dma_start(ts_w[k][:], TS_LHST[k])
            m1_w = [pts.tile([128, 512], f32, tag=f"m1w{k}", name=f"m1w{k}") for k in range(4)]
            for k in range(4):
                nc.sync.dma_start(m1_w[k][:], M1_LHST[k])
            m2_w = [pts.tile([128, 256], f32, tag=f"m2w{k}", name=f"m2w{k}") for k in range(4)]
            for k in range(4):
                nc.sync.dma_start(m2_w[k][:], M2_LHST[k])
            tsb = [pts.tile([128, 1], f32, tag=f"tsb{i}", name=f"tsb{i}") for i in range(14)]
            for i in range(14):
                nc.sync.dma_start(tsb[i][:], TS_BIAS[i])
            m1b = [pts.tile([128, 1], f32, tag=f"m1b{i}", name=f"m1b{i}") for i in range(4)]
            for i in range(4):
                nc.sync.dma_start(m1b[i][:], M1_BIAS[i])
            m2b = [pts.tile([128, 1], f32, tag=f"m2b{i}", name=f"m2b{i}") for i in range(2)]
            for i in range(2):
                nc.sync.dma_start(m2b[i][:], M2_BIAS[i])

            tcol = pts.tile([128, 4], f32, tag="tcol", name="tcol")
            nc.sync.dma_start(tcol[:], TCOL[:])
            st = pts.tile([128, 4], f32, tag="st", name="st")
            nc.scalar.activation(st[:], tcol[:], AF.Silu)

            tsv = [ctile(f"tsv{i}") for i in range(14)]
            for i in range(14):
                ps = psmv.tile([128, 1], f32, tag="mv", name="mv", bufs=2)
                for k in range(4):
                    nc.tensor.matmul(ps[:], ts_w[k][:, i * 128:(i + 1) * 128],
                                     st[:, k:k + 1], start=(k == 0), stop=(k == 3))
                TT(tsv[i][:], ps[:], tsb[i][:], ALU.add)

            ssum = [ctile(f"ssum{m}") for m in range(2)]
            for m in range(2):
                nc.vector.reduce_sum(ssum[m][:], X[m][:], AX.X)
            ssq = sumsq_cols(X, "ssq", pts, 2)
            mu_c, is_c = group_stats(ssum, ssq, "g", psmv)
            A_spc, B_spc = affine_coefs(gng, bng, mu_c, is_c, tsv[0:2], tsv[2:4], "spc")
            A_spr, B_spr = affine_coefs(gnl, bnl, mu_c, is_c, tsv[6:8], tsv[8:10], "spr")

            gap = [ctile(f"gap{m}") for m in range(2)]
            for m in range(2):
                mx = ctile(f"mx{m}")
                TS(mx[:], ssum[m][:], 1.0 / N, None, ALU.mult)
                TT(gap[m][:], A_spc[m][:], mx[:], ALU.mult)
                TT(gap[m][:], gap[m][:], B_spc[m][:], ALU.add)

            zcols = [gap[0], gap[1], tsv[4], tsv[5]]
            zh = [ctile(f"zh{i}") for i in range(4)]
            for i in range(4):
                ps = psmv.tile([128, 1], f32, tag="mv", name="mv", bufs=2)
                for k in range(4):
                    nc.tensor.matmul(ps[:], m1_w[k][:, i * 128:(i + 1) * 128],
                                     zcols[k][:], start=(k == 0), stop=(k == 3))
                nc.scalar.activation(zh[i][:], ps[:], AF.Silu, bias=m1b[i][:])
            prompt = [ctile(f"prompt{i}") for i in range(2)]
            for i in range(2):
                ps = psmv.tile([128, 1], f32, tag="mv", name="mv", bufs=2)
                for k in range(4):
                    nc.tensor.matmul(ps[:], m2_w[k][:, i * 128:(i + 1) * 128],
                                     zh[k][:], start=(k == 0), stop=(k == 3))
                nc.scalar.activation(prompt[i][:], ps[:], AF.Identity,
                                     bias=m2b[i][:])

        # hp_spr written BEFORE X is overwritten in-place with u = x + fg.
        # php stays open through phase C (dw conv reads it).
        s1 = [ctile(f"s1_{m}") for m in range(2)]
        with tc.tile_pool(name="php", bufs=1) as php:
            hp_spr = [php.tile([128, HP], f32r, tag=f"hp{m}", name=f"hp{m}")
                      for m in range(2)]
            for m in range(2):
                nc.gpsimd.memset(hp_spr[m][:].bitcast(f32), 0.0)
                dst = hp_spr[m][:].rearrange("p (r w) -> p r w", w=PW)[:, 1:65, 1:65]
                src = X[m][:].rearrange("p (r w) -> p r w", w=W)
                nc.scalar.activation(dst, src, AF.Identity,
                                     bias=B_spr[m][:], scale=A_spr[m][:])

            # ======= phase B: mask, h_spc, v, attention, proj, u=x+fg =======
            with tc.tile_pool(name="pattn", bufs=1) as pattn:
                maskc = [ctile(f"maskc{m}") for m in range(2)]
                maddc = [ctile(f"maddc{m}") for m in range(2)]
                for m in range(2):
                    nc.sync.dma_start(BOUNCE[0, m * 128:(m + 1) * 128], prompt[m][:])
                prow = pattn.tile([1, 256], f32, tag="prow", name="prow")
                nc.sync.dma_start(prow[:], BOUNCE[0][None, :])
                pb = pattn.tile([128, 256], f32, tag="pb", name="pb")
                nc.gpsimd.partition_broadcast(pb[:], prow[:])
                for m in range(2):
                    g = pattn.tile([128, 256], f32, tag="gcmp", name="gcmp", bufs=2)
                    TS(g[:], pb[:], prompt[m][:], None, ALU.is_gt)
                    e = pattn.tile([128, 256], f32, tag="ecmp", name="ecmp", bufs=2)
                    TS(e[:], pb[:], prompt[m][:], None, ALU.is_equal)
                    TT(e[:], e[:], ltri[m][:], ALU.mult)
                    TT(g[:], g[:], e[:], ALU.add)
                    rank = ctile(f"rank{m}")
                    nc.vector.reduce_sum(rank[:], g[:], AX.X)
                    TS(maskc[m][:], rank[:], float(TK), None, ALU.is_lt)
                    TS(maddc[m][:], maskc[m][:], 1e30, -1e30, ALU.mult, ALU.add)
                    nc.sync.dma_start(BOUNCE[1, m * 128:(m + 1) * 128], maddc[m][:])
                madd_row = pattn.tile([1, 256], f32r, tag="madd_row", name="madd_row")
                nc.gpsimd.dma_start(madd_row[:], BOUNCE[1][None, :])
                if DEBUG:
                    for m in range(2):
                        nc.sync.dma_start(DBG_PROMPT[m], prompt[m][:])
                        nc.sync.dma_start(DBG_MASK[m], maskc[m][:])

                attn_n = [pattn.tile([128, 256], f32r, tag=f"attnn{i}", name=f"attnn{i}")
                          for i in range(2)]
                attnT = [pattn.tile([128, 256], f32r, tag=f"attnT{j}", name=f"attnT{j}")
                         for j in range(2)]

                with tc.tile_pool(name="pv", bufs=1) as pv:
                    v = [pv.tile([128, N], f32r, tag=f"v{m}", name=f"v{m}") for m in range(2)]

                    with tc.tile_pool(name="phspc", bufs=1) as phspc:
                        hspc = [phspc.tile([128, N], f32r, tag=f"hspc{m}", name=f"hspc{m}")
                                for m in range(2)]
                        for m in range(2):
                            nc.scalar.activation(hspc[m][:], X[m][:], AF.Identity,
                                                 bias=B_spc[m][:], scale=A_spc[m][:])

                        with tc.tile_pool(name="psv", bufs=2, space="PSUM") as psv:
                            for m in range(2):
                                for half in range(2):
                                    ps = psv.tile([128, 2048], f32, tag="vps", name="vps")
                                    for s in range(4):
                                        n0 = half * 2048 + s * 512
                                        for k in range(2):
                                            nc.tensor.matmul(
                                                ps[:, s * 512:(s + 1) * 512],
                                                wv[k][:, m * 128:(m + 1) * 128],
                                                hspc[k][:, n0:n0 + 512],
                                                start=(k == 0), stop=(k == 1))
                                    nc.scalar.activation(
                                        v[m][:, half * 2048:(half + 1) * 2048], ps[:],
                                        AF.Identity, bias=bv[m][:])

                        with tc.tile_pool(name="pqk", bufs=3) as pqk, \
                             tc.tile_pool(name="psqk", bufs=1, space="PSUM") as psqk, \
                             tc.tile_pool(name="psattn", bufs=1, space="PSUM") as psattn:
                            ps_attn = [psattn.tile([128, 256], f32, tag=f"attn{i}", name=f"attn{i}")
                                       for i in range(2)]
                            for mi in range(32):
                                n0 = mi * 128
                                ps_q = psqk.tile([128, 256], f32, tag="q", name="q", bufs=2)
                                ps_k = psqk.tile([128, 256], f32, tag="k", name="k", bufs=2)
                                for k in range(2):
                                    nc.tensor.matmul(ps_q[:], hspc[k][:, n0:n0 + 128],
                                                     wq[k][:], start=(k == 0), stop=False)
                                nc.tensor.matmul(ps_q[:], ones128[:], bq_row[:],
                                                 start=False, stop=True)
                                for k in range(2):
                                    nc.tensor.matmul(ps_k[:], hspc[k][:, n0:n0 + 128],
                                                     wk[k][:], start=(k == 0), stop=False)
                                nc.tensor.matmul(ps_k[:], ones128[:], bk_row[:],
                                                 start=False, stop=True)
                                qt = pqk.tile([128, 256], f32r, tag="qt", name="qt")
                                nc.vector.tensor_copy(qt[:], ps_q[:])
                                kt = pqk.tile([128, 256], f32r, tag="kt", name="kt")
                                nc.scalar.copy(kt[:], ps_k[:])
                                for i in range(2):
                                    nc.tensor.matmul(ps_attn[i][:],
                                                     qt[:, i * 128:(i + 1) * 128], kt[:],
                                                     start=(mi == 0), stop=False)
                            for i in range(2):
                                nc.tensor.matmul(ps_attn[i][:], ones128[:], madd_row[:],
                                                 start=False, stop=True)

                            for i in range(2):
                                mxv = ctile(f"smmax{i}")
                                nc.vector.reduce_max(mxv[:], ps_attn[i][:], AX.X)
                                nmx = ctile(f"smnmx{i}")
                                TS(nmx[:], mxv[:], -SCALE, None, ALU.mult)
                                expo = pqk.tile([128, 256], f32, tag="expo", name="expo")
                                nc.scalar.activation(expo[:], ps_attn[i][:], AF.Exp,
                                                     bias=nmx[:], scale=SCALE)
                                rs = ctile(f"smsum{i}")
                                nc.vector.reduce_sum(rs[:], expo[:], AX.X)
                                rec = ctile(f"smrec{i}")
                                nc.vector.reciprocal(rec[:], rs[:])
                                TS(attn_n[i][:], expo[:], rec[:], maskc[i][:],
                                   ALU.mult, ALU.mult)
                                if DEBUG:
                                    nc.sync.dma_start(DBG_ATTN[i], attn_n[i][:].bitcast(f32))

                            for i in range(2):
                                for j in range(2):
                                    ps_t = psqk.tile([128, 128], f32r, tag="tr", name="tr", bufs=2)
                                    nc.tensor.transpose(ps_t[:],
                                                        attn_n[i][:, j * 128:(j + 1) * 128],
                                                        ident[:])
                                    nc.scalar.activation(attnT[j][:, i * 128:(i + 1) * 128],
                                                         ps_t[:].bitcast(f32), AF.Identity)

                    with tc.tile_pool(name="posa", bufs=1) as posa, \
                         tc.tile_pool(name="psconv", bufs=2, space="PSUM") as psconv:
                        # per 2048-column half: osa for both k-chunks, then the
                        # proj + residual add for both m-chunks (small rotating
                        # tiles instead of full [C,N] buffers).
                        for half in range(2):
                            osah = []
                            for k in range(2):
                                ps = psconv.tile([128, 2048], f32, tag="cps", name="cps")
                                for s in range(4):
                                    n0 = half * 2048 + s * 512
                                    for j in range(2):
                                        nc.tensor.matmul(
                                            ps[:, s * 512:(s + 1) * 512],
                                            attnT[j][:, k * 128:(k + 1) * 128],
                                            v[j][:, n0:n0 + 512],
                                            start=(j == 0), stop=(j == 1))
                                ot = posa.tile([128, 2048], f32r, tag=f"osa{k}",
                                               name=f"osa{k}")
                                nc.vector.tensor_copy(ot[:], ps[:])
                                osah.append(ot)
                            for m in range(2):
                                ps = psconv.tile([128, 2048], f32, tag="cps", name="cps")
                                for s in range(4):
                                    for k in range(2):
                                        nc.tensor.matmul(
                                            ps[:, s * 512:(s + 1) * 512],
                                            wp[k][:, m * 128:(m + 1) * 128],
                                            osah[k][:, s * 512:(s + 1) * 512],
                                            start=(k == 0), stop=(k == 1))
                                fgh = posa.tile([128, 2048], f32, tag="fgh",
                                                name="fgh", bufs=2)
                                nc.scalar.activation(fgh[:], ps[:], AF.Identity,
                                                     bias=bp[m][:])
                                if DEBUG:
                                    nc.sync.dma_start(
                                        DBG_FG[m][:, half * 2048:(half + 1) * 2048],
                                        fgh[:])
                                # u = x + fg, in place into X
                                TT(X[m][:, half * 2048:(half + 1) * 2048],
                                   X[m][:, half * 2048:(half + 1) * 2048],
                                   fgh[:], ALU.add)

            # ======= phase C: dw + pw, x1 = u + w2r*fl ======================
            with tc.tile_pool(name="pdwo", bufs=1) as pdwo, \
                 tc.tile_pool(name="psconv2", bufs=2, space="PSUM") as psconv2:
                dwo = [pdwo.tile([128, N], f32r, tag=f"dwo{m}", name=f"dwo{m}")
                       for m in range(2)]
                for m in range(2):
                    hview = hp_spr[m][:].rearrange("p (r w) -> p r w", w=PW)
                    for half in range(2):
                        ps = psconv2.tile([128, 2048], f32, tag="cps2", name="cps2")
                        for s in range(4):
                            r0 = half * 32 + s * 8
                            first = True
                            for kh in range(3):
                                for kw in range(3):
                                    tap = hview[:, r0 + kh:r0 + kh + 8, kw:kw + 64]
                                    nc.tensor.matmul(
                                        ps[:, s * 512:(s + 1) * 512],
                                        dwd[kh * 3 + kw][m][:], tap,
                                        start=first, stop=(kh == 2 and kw == 2))
                                    first = False
                        nc.scalar.activation(
                            dwo[m][:, half * 2048:(half + 1) * 2048], ps[:],
                            AF.Silu, bias=bdw[m][:])

                with tc.tile_pool(name="pfl", bufs=1) as pfl:
                    fl = [pfl.tile([128, N], f32, tag=f"fl{m}", name=f"fl{m}")
                          for m in range(2)]
                    for m in range(2):
                        for half in range(2):
                            ps = psconv2.tile([128, 2048], f32, tag="cps2", name="cps2")
                            for s in range(4):
                                n0 = half * 2048 + s * 512
                                for k in range(2):
                                    nc.tensor.matmul(
                                        ps[:, s * 512:(s + 1) * 512],
                                        wpw[k][:, m * 128:(m + 1) * 128],
                                        dwo[k][:, n0:n0 + 512],
                                        start=(k == 0), stop=(k == 1))
                            nc.scalar.activation(
                                fl[m][:, half * 2048:(half + 1) * 2048], ps[:],
                                AF.Silu, bias=bpw[m][:])
                    if DEBUG:
                        for m in range(2):
                            nc.sync.dma_start(DBG_FL[m], fl[m][:])

                    for m in range(2):
                        TS(fl[m][:], fl[m][:], w2r[m][:], None, ALU.mult)
                        TT(X[m][:], X[m][:], fl[m][:], ALU.add)
                        nc.vector.reduce_sum(s1[m][:], X[m][:], AX.X)
                    if DEBUG:
                        for m in range(2):
                            nc.sync.dma_start(DBG_X1[m], X[m][:])

        # ======= phase D: GN-f stats + msgn =================================
        x1 = X
        with tc.tile_pool(name="pwa", bufs=1) as pwa, \
             tc.tile_pool(name="phm", bufs=1) as phm:
            with tc.tile_pool(name="psm", bufs=1, space="PSUM") as psm:
                q1v = sumsq_cols(x1, "q1c", phm, 1)
                mu1, is1 = group_stats(s1, q1v, "f", psm)
            psm3 = top.enter_context(tc.tile_pool(name="psm3", bufs=2, space="PSUM"))
            A_m, B_m = affine_coefs(gnf, bnf, mu1, is1, tsv[10:12], tsv[12:14], "f")

            wa = [[wt(pwa, f"wa{t_}_{k}", WA_T[t_, k], [128, 256], f32r) for k in range(2)]
                  for t_ in range(9)]

            hp_m = [phm.tile([128, HP], f32r, tag=f"hpm{m}", name=f"hpm{m}")
                    for m in range(2)]
            for m in range(2):
                nc.gpsimd.memset(hp_m[m][:].bitcast(f32), 0.0)
                dst = hp_m[m][:].rearrange("p (r w) -> p r w", w=PW)[:, 1:65, 1:65]
                src = x1[m][:].rearrange("p (r w) -> p r w", w=W)
                nc.scalar.activation(dst, src, AF.Identity,
                                     bias=B_m[m][:], scale=A_m[m][:])

            hview = [hp_m[k][:].rearrange("p (r w) -> p r w", w=PW) for k in range(2)]

            sb = [phm.tile([128, N], f32, tag=f"sb{m}", name=f"sb{m}") for m in range(2)]
            for m in range(2):
                for half in range(2):
                    ps = psm3.tile([128, 2048], f32, tag="cps3", name="cps3")
                    for s in range(4):
                        r0_ = half * 32 + s * 8
                        for k in range(2):
                            tap = hview[k][:, r0_ + 1:r0_ + 9, 1:65]
                            nc.tensor.matmul(ps[:, s * 512:(s + 1) * 512],
                                             wb[k][:, m * 128:(m + 1) * 128], tap,
                                             start=(k == 0), stop=(k == 1))
                    nc.scalar.activation(sb[m][:, half * 2048:(half + 1) * 2048],
                                         ps[:], AF.Sigmoid, bias=bb[m][:])

            for m in range(2):
                for half in range(2):
                    ps = psm3.tile([128, 2048], f32, tag="cps3", name="cps3")
                    for s in range(4):
                        r0_ = half * 32 + s * 8
                        out_sl = ps[:, s * 512:(s + 1) * 512]
                        nc.tensor.matmul(out_sl, ba_row[m][:], ones512[:],
                                         start=True, stop=False)
                        for kh in range(3):
                            for kw in range(3):
                                for k in range(2):
                                    tap = hview[k][:, r0_ + kh:r0_ + kh + 8, kw:kw + 64]
                                    nc.tensor.matmul(
                                        out_sl,
                                        wa[kh * 3 + kw][k][:, m * 128:(m + 1) * 128],
                                        tap, start=False,
                                        stop=(kh == 2 and kw == 2 and k == 1))
                    h0 = half * 2048
                    gsc = phm.tile([128, 2048], f32, tag="gsc", name="gsc", bufs=1)
                    TT(gsc[:], ps[:], sb[m][:, h0:h0 + 2048], ALU.mult)
                    yt = phm.tile([128, 2048], f32, tag="yt", name="yt", bufs=1)
                    TT(yt[:], x1[m][:, h0:h0 + 2048], gsc[:], ALU.add)
                    nc.sync.dma_start(Y[m][:, h0:h0 + 2048], yt[:])

    nc.compile()
    return nc


def _prep_shared(params):
    p = {k: np.asarray(v, dtype=np.float32) for k, v in params.items()}
    d = {}

    w_all = np.concatenate([p["spc_ts_W"], p["spc_tp_W"], p["spr_ts_W"],
                            p["msgn_ts_W"]], axis=0)          # [1792, 512]
    d["ts_lhsT"] = np.ascontiguousarray(w_all.T).reshape(4, 128, 1792)
    b_all = np.concatenate([p["spc_ts_b"], p["spc_tp_b"], p["spr_ts_b"],
                            p["msgn_ts_b"]])
    d["ts_bias"] = b_all.reshape(14, 128, 1)
    d["m1_lhsT"] = np.ascontiguousarray(p["spc_m1_W"].T).reshape(4, 128, 512)
    d["m1_bias"] = p["spc_m1_b"].reshape(4, 128, 1)
    d["m2_lhsT"] = np.ascontiguousarray(p["spc_m2_W"].T).reshape(4, 128, 256)
    d["m2_bias"] = p["spc_m2_b"].reshape(2, 128, 1)

    def t2(w):
        return np.ascontiguousarray(w.T).reshape(2, 128, 256)

    w1r = p["aafm_w1r"][0, :, 0, 0]
    w2r = p["aafm_w2r"][0, :, 0, 0]
    d["wq_t"] = t2(p["spc_q_w"][:, :, 0, 0])
    d["wk_t"] = t2(p["spc_k_w"][:, :, 0, 0])
    d["wv_t"] = t2(p["spc_v_w"][:, :, 0, 0])
    d["wp_t"] = t2(w1r[:, None] * p["spc_proj_w"][:, :, 0, 0])
    d["wpw_t"] = t2(p["spr_pw_w"][:, :, 0, 0])
    d["wb_t"] = t2(p["msgn_b_w"][:, :, 0, 0])
    wa = np.empty((9, 2, 128, 256), np.float32)
    for kh in range(3):
        for kw in range(3):
            wa[kh * 3 + kw] = t2(p["msgn_a_w"][:, :, kh, kw])
    d["wa_t"] = wa
    dwd = np.zeros((9, 2, 128, 128), np.float32)
    for kh in range(3):
        for kw in range(3):
            for c in range(2):
                np.fill_diagonal(dwd[kh * 3 + kw, c],
                                 p["spr_dw_w"][c * 128:(c + 1) * 128, 0, kh, kw])
    d["dw_diag"] = dwd
    d["bq_row"] = p["spc_q_b"].reshape(1, 256)
    d["bk_row"] = p["spc_k_b"].reshape(1, 256)
    d["bv_col"] = p["spc_v_b"].reshape(2, 128, 1)
    d["bp_col"] = (w1r * p["spc_proj_b"]).reshape(2, 128, 1)
    d["bpw_col"] = p["spr_pw_b"].reshape(2, 128, 1)
    d["bb_col"] = p["msgn_b_b"].reshape(2, 128, 1)
    d["ba_row"] = p["msgn_a_b"].reshape(2, 1, 128)
    d["bdw_col"] = p["spr_dw_b"].reshape(2, 128, 1)
    d["w2r_col"] = w2r.reshape(2, 128, 1)
    d["g_ng"] = p["ng_g"].reshape(2, 128, 1)
    d["b_ng"] = p["ng_b"].reshape(2, 128, 1)
    d["g_nl"] = p["nl_g"].reshape(2, 128, 1)
    d["b_nl"] = p["nl_b"].reshape(2, 128, 1)
    d["g_nf"] = p["nf_g"].reshape(2, 128, 1)
    d["b_nf"] = p["nf_b"].reshape(2, 128, 1)

    G = np.zeros((GROUPS, C), np.float32)
    for c in range(C):
        G[c // CPG, c] = 1.0
    d["gmat_t"] = np.ascontiguousarray(G.T).reshape(2, 128, GROUPS)
    d["gmat"] = G
    i_glob = np.arange(C)[:, None]
    j = np.arange(C)[None, :]
    d["ltri"] = (j < i_glob).astype(np.float32).reshape(2, 128, 256)
    d["ident"] = np.eye(128, dtype=np.float32)
    d["ones1x128"] = np.ones((1, 128), np.float32)
    d["ones1x512"] = np.ones((1, 512), np.float32)
    return d


def kernel(x, temb, params):
    from concourse.bass_utils import run_bass_kernel_spmd

    if "nc" not in _CACHE:
        _CACHE["nc"] = _build()
    nc = _CACHE["nc"]

    x = np.asarray(x, dtype=np.float32)
    temb = np.asarray(temb, dtype=np.float32)
    shared = _prep_shared(params)
    in_maps = []
    for b in range(NCORES):
        m = dict(shared)
        m["xb"] = np.ascontiguousarray(x[b].reshape(2, 128, N))
        m["temb_col"] = np.ascontiguousarray(temb[b].reshape(4, 128).T)
        in_maps.append(m)
    res = run_bass_kernel_spmd(nc, in_maps, list(range(NCORES)))
    out = np.stack([res.results[b]["y"].reshape(C, H, W) for b in range(NCORES)])
    if DEBUG:
        kernel._dbg = res.results
    return out
